# revision 1
# baseline (speedup 1.0000x reference)
"""DNeRF-TensoRF sampler kernel for Trainium2 (8 NeuronCores, data-parallel over points).

Strategy:
  - Host pre-packs the 9 feature planes (3 feats x 3 components) into 3 gather
    tables PT_j of shape (NUM_FRAMES*RESO, 384) fp32 where row r = t*RESO+x holds
    [A | D]: A[f*64+c] = feat_f[j,c,t,x], D = A(next x) - A  (difference table, so
    the device lerp is A + wx*D).  The y (time) coordinate is an exact integer
    frame index, so the bilinear reduces to a 1-D lerp along x at row t.
  - x is sharded over 8 cores along the point axis.  Each core processes
    PC = P/8 points: computes int16 row indices on 16 partitions (the layout
    dma_gather needs), gathers 1536B rows per (point, component) with SWDGE
    dma_gather, lerps, multiplies the 3 component samples, reduces over 64
    channels, and evaluates the sin/cos positional encoding on the ACT engine.
  - Outputs are written in an interleaved device layout and un-permuted on host.
"""
import sys

sys.path.insert(0, "/opt/trn_rl_repo")

from contextlib import ExitStack

import numpy as np

import concourse.bacc as bacc
import concourse.bass as bass
import concourse.mybir as mybir
import concourse.tile as tile
from concourse.bass_utils import run_bass_kernel_spmd

NUM_FRAMES = 100
RESO = 256
CHAN = 64
FREQ = 10
P = 524288
NCORES = 8
PC = P // NCORES            # 65536 points per core
NROWS = NUM_FRAMES * RESO   # 25600 table rows
ES = 2 * 3 * CHAN           # 384 floats per gather row ([A|D], 3 feats x 64 ch)
NB = 1024                   # points per gather batch
K_RED = 64 * np.pi          # even multiple of 2*pi used to make mod args positive

TWO_PI = float(2 * np.pi)
INV_TWO_PI = float(1.0 / (2 * np.pi))
# largest fp32 strictly below float64 pi (Sin activation domain is [-pi, pi])
PI_LO = float(np.nextafter(np.float32(np.pi), np.float32(0.0)))


def build_program(pc=PC, nb=NB, hw_round=True, reps=1, pool_prod=False):
    """hw_round: HW casts fp32->int with round-to-nearest; CoreSim truncates.
    floor(v) is computed as cast(v + CAST_OFF) with CAST_OFF=-0.5 on HW, 0 in
    sim; round(v) as cast(v + RED_OFF) with RED_OFF=0 on HW, +0.5 in sim.
    reps: repeat the main loop (timing amplification).  pool_prod: run the
    first product multiply on GPSIMD to offload the vector engine."""
    cast_off = -0.5 if hw_round else 0.0
    red_off = 0.0 if hw_round else 0.5
    m = nb // 128           # free slots per batch
    lw = pc // 16           # idx-layout free size
    mc = pc // 128          # interleaved-layout free size
    nbatch = pc // nb
    f32 = mybir.dt.float32
    i16 = mybir.dt.int16
    i32 = mybir.dt.int32
    A = mybir.AluOpType

    nc = bacc.Bacc("TRN2", target_bir_lowering=False, debug=False)

    xw = nc.dram_tensor("xw", [16, 4 * lw], f32, kind="ExternalInput")
    xc = nc.dram_tensor("xc", [128, mc * 4], f32, kind="ExternalInput")
    pts = [
        nc.dram_tensor(f"pt{j}", [NROWS, ES], f32, kind="ExternalInput")
        for j in range(3)
    ]
    fr = nc.dram_tensor("fr", [128, 30], f32, kind="ExternalInput")
    out = nc.dram_tensor("out", [128, mc * 63], f32, kind="ExternalOutput")

    with tile.TileContext(nc) as tc, ExitStack() as ctx:
        cpool = ctx.enter_context(tc.tile_pool(name="const", bufs=1))
        frt = cpool.tile([128, 30], f32)
        nc.sync.dma_start(frt[:], fr.ap()[:])
        xct = cpool.tile([128, mc * 4], f32)
        nc.sync.dma_start(xct[:], xc.ap()[:])
        wxt = cpool.tile([128, 3 * mc], f32)
        idx128 = cpool.tile([128, 3 * lw], i16)

        # ---- setup: per-point row indices on 16 partitions, weights on 128 ----
        with tc.tile_pool(name="setup", bufs=1) as spool:
            xwt = spool.tile([16, 4 * lw], f32)
            nc.sync.dma_start(xwt[:], xw.ap()[:])
            xw_t = xwt[:, 3 * lw:4 * lw]
            for j in range(3):
                xw_j = xwt[:, j * lw:(j + 1) * lw]
                ixw = spool.tile([16, lw], f32, tag="ixw")
                nc.vector.tensor_scalar(ixw[:], xw_j, 255.0, cast_off,
                                        A.mult, A.add)
                nc.vector.tensor_scalar(ixw[:], ixw[:], 0.0, None, A.max)
                ixi = spool.tile([16, lw], i32, tag="ixi")
                nc.vector.tensor_copy(ixi[:], ixw[:])   # == floor(255*x)
                x0w = spool.tile([16, lw], f32, tag="x0w")
                nc.vector.tensor_copy(x0w[:], ixi[:])
                rw = spool.tile([16, lw], f32, tag="rw")
                nc.vector.scalar_tensor_tensor(
                    rw[:], xw_t, 256.0, x0w[:], A.mult, A.add)
                idx16 = spool.tile([16, lw], i16, tag="idx16")
                nc.vector.tensor_copy(idx16[:], rw[:])
                for k in range(8):
                    nc.sync.dma_start(
                        idx128[16 * k:16 * (k + 1), j * lw:(j + 1) * lw], idx16[:])
                # interleaved-layout lerp weights: wx1 = ix - floor(ix)
                xj = xct[:].rearrange("p (q f) -> p q f", f=4)[:, :, j]
                ixc = spool.tile([128, mc], f32, tag="ixc")
                nc.vector.tensor_scalar(ixc[:], xj, 255.0, None, A.mult)
                icm = spool.tile([128, mc], f32, tag="icm")
                nc.vector.tensor_scalar(icm[:], xj, 255.0, cast_off,
                                        A.mult, A.add)
                nc.vector.tensor_scalar(icm[:], icm[:], 0.0, None, A.max)
                ici = spool.tile([128, mc], i32, tag="ici")
                nc.vector.tensor_copy(ici[:], icm[:])
                icf = spool.tile([128, mc], f32, tag="icf")
                nc.vector.tensor_copy(icf[:], ici[:])
                nc.vector.tensor_tensor(
                    wxt[:, j * mc:(j + 1) * mc], ixc[:], icf[:], A.subtract)

        xyz = xct[:].rearrange("p (q f) -> p q f", f=4)[:, :, 0:3]  # (128, mc, 3)

        gpool = ctx.enter_context(tc.tile_pool(name="g", bufs=2))
        tpool = ctx.enter_context(tc.tile_pool(name="t", bufs=2))
        opool = ctx.enter_context(tc.tile_pool(name="o", bufs=2))

        out_v = out.ap().rearrange("p (q k) -> p q k", k=63)

        for b in [bb for _ in range(reps) for bb in range(nbatch)]:
            gs = []
            for j in range(3):
                g = gpool.tile([128, m, ES], f32, tag=f"g{j}")
                nc.gpsimd.dma_gather(
                    g[:], pts[j].ap()[:],
                    idx128[:, j * lw + b * (nb // 16): j * lw + (b + 1) * (nb // 16)],
                    nb, nb, ES)
                gs.append(g)
            ss = []
            for j in range(3):
                wb = (wxt[:, j * mc + b * m: j * mc + (b + 1) * m]
                      .unsqueeze(2).to_broadcast([128, m, 192]))
                mt = tpool.tile([128, m, 192], f32, tag="mt", bufs=3)
                nc.vector.tensor_tensor(mt[:], gs[j][:, :, 192:384], wb, A.mult)
                st = tpool.tile([128, m, 192], f32, tag=f"s{j}")
                nc.vector.tensor_tensor(st[:], mt[:], gs[j][:, :, 0:192], A.add)
                ss.append(st)
            p01 = tpool.tile([128, m, 192], f32, tag="p01")
            prod_eng = nc.gpsimd if pool_prod else nc.vector
            prod_eng.tensor_tensor(p01[:], ss[0][:], ss[1][:], A.mult)
            pr = tpool.tile([128, m, 192], f32, tag="pr")
            nc.vector.tensor_tensor(pr[:], p01[:], ss[2][:], A.mult)
            delta = tpool.tile([128, m, 3], f32, tag="delta")
            nc.vector.tensor_reduce(
                delta[:], pr[:].rearrange("p q (f c) -> p q f c", c=CHAN),
                mybir.AxisListType.X, A.add)
            pxyz = tpool.tile([128, m, 3], f32, tag="pxyz")
            nc.vector.tensor_tensor(
                pxyz[:], delta[:], xyz[:, b * m:(b + 1) * m, :], A.add)

            ot = opool.tile([128, m, 63], f32, tag="ot")
            nc.scalar.activation(
                ot[:, :, 0:3], pxyz[:], mybir.ActivationFunctionType.Copy)

            ang = tpool.tile([128, m, 30], f32, tag="ang")
            nc.vector.tensor_tensor(
                ang[:].rearrange("p q (k j) -> p q k j", j=3),
                pxyz[:].unsqueeze(2).to_broadcast([128, m, FREQ, 3]),
                frt[:].rearrange("p (k j) -> p k j", j=3)
                      .unsqueeze(1).to_broadcast([128, m, FREQ, 3]),
                A.mult)
            # range reduction: w = angK - 2*pi*round(angK / 2*pi)  in [-pi, pi]
            sc_out = ot[:, :, 3:63].rearrange("p q (k s j) -> p q k s j", s=2, j=3)
            angK = tpool.tile([128, m, 30], f32, tag="angK")
            nc.vector.tensor_scalar(angK[:], ang[:], float(K_RED), None, A.add)
            for s, phase in ((0, 0.0), (1, float(np.pi / 2))):
                az = tpool.tile([128, m, 30], f32, tag="az")
                if phase:
                    nc.vector.tensor_scalar(az[:], angK[:], phase, None, A.add)
                    src = az
                else:
                    src = angK
                z = tpool.tile([128, m, 30], f32, tag="z")
                nc.vector.tensor_scalar(
                    z[:], src[:], INV_TWO_PI, red_off, A.mult, A.add)
                zi = tpool.tile([128, m, 30], i32, tag="zi")
                nc.vector.tensor_copy(zi[:], z[:])
                zf = tpool.tile([128, m, 30], f32, tag="zf")
                nc.vector.tensor_copy(zf[:], zi[:])
                wred = tpool.tile([128, m, 30], f32, tag="wred")
                nc.vector.scalar_tensor_tensor(
                    wred[:], zf[:], -TWO_PI, src[:], A.mult, A.add)
                wcl = tpool.tile([128, m, 30], f32, tag="wcl")
                nc.vector.tensor_scalar(wcl[:], wred[:], -PI_LO, None, A.max)
                nc.vector.tensor_scalar(wcl[:], wcl[:], PI_LO, None, A.min)
                nc.scalar.activation(
                    sc_out[:, :, :, s, :],
                    wcl[:].rearrange("p q (k j) -> p q k j", j=3),
                    mybir.ActivationFunctionType.Sin)

            nc.sync.dma_start(out_v[:, b * m:(b + 1) * m, :], ot[:])

    nc.compile()
    return nc


def pack_tables(feat0, feat1, feat2):
    """Build the 3 per-component gather tables (NROWS, 384) fp32 [A | D]."""
    pts = []
    for j in range(3):
        planes = np.stack([feat0[j], feat1[j], feat2[j]], axis=0)  # (3,64,100,256)
        a = np.ascontiguousarray(
            planes.transpose(2, 3, 0, 1).reshape(NROWS, 3 * CHAN)).astype(np.float32)
        d = np.zeros_like(a)
        d[:-1] = a[1:] - a[:-1]
        d[RESO - 1::RESO] = 0.0  # x=255 rows never used as base; avoid cross-frame
        pts.append(np.concatenate([a, d], axis=1))
    return pts


def pack_x(x_shard):
    """x_shard (PC,4) -> (xw (16,4*LW), xc (128,MC*4))."""
    pc = x_shard.shape[0]
    lw, mc = pc // 16, pc // 128
    xw = np.concatenate(
        [np.ascontiguousarray(x_shard[:, j].reshape(lw, 16).T) for j in range(4)],
        axis=1).astype(np.float32)
    xc = np.ascontiguousarray(
        x_shard.reshape(mc, 128, 4).transpose(1, 0, 2).reshape(128, mc * 4)
    ).astype(np.float32)
    return xw, xc


_NC_CACHE = {}


def kernel(x, feat0, feat1, feat2):
    x = np.asarray(x, dtype=np.float32)
    feat0 = np.asarray(feat0, dtype=np.float32)
    feat1 = np.asarray(feat1, dtype=np.float32)
    feat2 = np.asarray(feat2, dtype=np.float32)

    if "nc" not in _NC_CACHE:
        _NC_CACHE["nc"] = build_program()
    nc = _NC_CACHE["nc"]

    pts = pack_tables(feat0, feat1, feat2)
    fr = np.tile(np.repeat(2.0 ** np.arange(FREQ), 3).astype(np.float32)[None, :],
                 (128, 1))

    in_maps = []
    for k in range(NCORES):
        xw, xc = pack_x(x[k * PC:(k + 1) * PC])
        in_maps.append({
            "xw": xw, "xc": xc,
            "pt0": pts[0], "pt1": pts[1], "pt2": pts[2],
            "fr": fr,
        })

    res = run_bass_kernel_spmd(nc, in_maps, core_ids=list(range(NCORES)))
    outs = []
    for k in range(NCORES):
        o = res.results[k]["out"].reshape(128, PC // 128, 63)
        outs.append(o.transpose(1, 0, 2).reshape(PC, 63))
    return np.concatenate(outs, axis=0)



# revision 4
# speedup vs baseline: 1.1392x; 1.1392x over previous
"""DNeRF-TensoRF sampler kernel for Trainium2 (8 NeuronCores, data-parallel over points).

Strategy:
  - Host pre-packs the 9 feature planes (3 feats x 3 components) into 3 gather
    tables PT_j of shape (NUM_FRAMES*RESO, 384) fp16 where row r = t*RESO+x holds
    [A | D]: A[f*64+c] = feat_f[j,c,t,x], D = A(next x) - A  (difference table, so
    the device lerp is A + wx*D).  The y (time) coordinate is an exact integer
    frame index, so the bilinear reduces to a 1-D lerp along x at row t.
  - x is sharded over 8 cores along the point axis.  Each core processes
    PC = P/8 points: computes int16 row indices on 16 partitions (the layout
    dma_gather needs), gathers 768B fp16 rows per (point, component) with SWDGE
    dma_gather, lerps/multiplies in fp16, reduces over 64 channels in fp32, and
    evaluates the sin/cos positional encoding via one ACT-engine sin/cos pair
    at the base frequency plus a double-angle recurrence on the vector engine
    (the FREQ frequencies are exactly 2^k, so sin/cos at level k come from
    level k-1 with 3 small ops instead of a full range reduction).
  - Outputs are written in an interleaved device layout and un-permuted on host.
"""
import sys

sys.path.insert(0, "/opt/trn_rl_repo")

from contextlib import ExitStack

import numpy as np

import concourse.bacc as bacc
import concourse.bass as bass
import concourse.mybir as mybir
import concourse.tile as tile
from concourse.bass_utils import run_bass_kernel_spmd

NUM_FRAMES = 100
RESO = 256
CHAN = 64
FREQ = 10
P = 524288
NCORES = 8
PC = P // NCORES            # 65536 points per core
NROWS = NUM_FRAMES * RESO   # 25600 table rows
ES = 2 * 3 * CHAN           # 384 fp16 per gather row ([A|D], 3 feats x 64 ch)
NB = 1024                   # points per gather batch

HALF_PI = float(np.pi / 2)


def build_program(pc=PC, nb=NB, hw_round=True, reps=1):
    """hw_round: HW casts fp32->int with round-to-nearest; CoreSim truncates.
    floor(v) is computed as cast(v + CAST_OFF) with CAST_OFF=-0.5 on HW, 0 in
    sim.  reps: repeat the main loop (timing amplification)."""
    cast_off = -0.5 if hw_round else 0.0
    m = nb // 128           # free slots per batch
    lw = pc // 16           # idx-layout free size
    mc = pc // 128          # interleaved-layout free size
    nbatch = pc // nb
    f32 = mybir.dt.float32
    f16 = mybir.dt.float16
    i16 = mybir.dt.int16
    i32 = mybir.dt.int32
    A = mybir.AluOpType
    ACT = mybir.ActivationFunctionType

    nc = bacc.Bacc("TRN2", target_bir_lowering=False, debug=False)

    xw = nc.dram_tensor("xw", [16, 4 * lw], f32, kind="ExternalInput")
    xc = nc.dram_tensor("xc", [128, mc * 4], f32, kind="ExternalInput")
    pts = [
        nc.dram_tensor(f"pt{j}", [NROWS, ES], f16, kind="ExternalInput")
        for j in range(3)
    ]
    out = nc.dram_tensor("out", [128, mc * 63], f32, kind="ExternalOutput")

    with tile.TileContext(nc) as tc, ExitStack() as ctx:
        cpool = ctx.enter_context(tc.tile_pool(name="const", bufs=1))
        hpi = cpool.tile([128, 1], f32)
        nc.vector.memset(hpi[:], HALF_PI)
        xct = cpool.tile([128, mc * 4], f32)
        nc.sync.dma_start(xct[:], xc.ap()[:])
        wxt = cpool.tile([128, 3 * mc], f32)
        wt16 = cpool.tile([128, 3 * mc], f16)
        idx128 = cpool.tile([128, 3 * lw], i16)

        # ---- setup: per-point row indices on 16 partitions, weights on 128 ----
        with tc.tile_pool(name="setup", bufs=1) as spool:
            xwt = spool.tile([16, 4 * lw], f32)
            nc.sync.dma_start(xwt[:], xw.ap()[:])
            xw_t = xwt[:, 3 * lw:4 * lw]
            for j in range(3):
                xw_j = xwt[:, j * lw:(j + 1) * lw]
                ixw = spool.tile([16, lw], f32, tag="ixw")
                nc.vector.tensor_scalar(ixw[:], xw_j, 255.0, cast_off,
                                        A.mult, A.add)
                nc.vector.tensor_scalar(ixw[:], ixw[:], 0.0, None, A.max)
                ixi = spool.tile([16, lw], i32, tag="ixi")
                nc.vector.tensor_copy(ixi[:], ixw[:])   # == floor(255*x)
                x0w = spool.tile([16, lw], f32, tag="x0w")
                nc.vector.tensor_copy(x0w[:], ixi[:])
                rw = spool.tile([16, lw], f32, tag="rw")
                nc.vector.scalar_tensor_tensor(
                    rw[:], xw_t, 256.0, x0w[:], A.mult, A.add)
                idx16 = spool.tile([16, lw], i16, tag="idx16")
                nc.vector.tensor_copy(idx16[:], rw[:])
                for k in range(8):
                    nc.sync.dma_start(
                        idx128[16 * k:16 * (k + 1), j * lw:(j + 1) * lw], idx16[:])
                # interleaved-layout lerp weights: wx1 = ix - floor(ix)
                xj = xct[:].rearrange("p (q f) -> p q f", f=4)[:, :, j]
                ixc = spool.tile([128, mc], f32, tag="ixc")
                nc.vector.tensor_scalar(ixc[:], xj, 255.0, None, A.mult)
                icm = spool.tile([128, mc], f32, tag="icm")
                nc.vector.tensor_scalar(icm[:], xj, 255.0, cast_off,
                                        A.mult, A.add)
                nc.vector.tensor_scalar(icm[:], icm[:], 0.0, None, A.max)
                ici = spool.tile([128, mc], i32, tag="ici")
                nc.vector.tensor_copy(ici[:], icm[:])
                icf = spool.tile([128, mc], f32, tag="icf")
                nc.vector.tensor_copy(icf[:], ici[:])
                nc.vector.tensor_tensor(
                    wxt[:, j * mc:(j + 1) * mc], ixc[:], icf[:], A.subtract)
            nc.vector.tensor_copy(wt16[:], wxt[:])  # fp16 weights for the lerp

        xyz = xct[:].rearrange("p (q f) -> p q f", f=4)[:, :, 0:3]  # (128, mc, 3)

        gpool = ctx.enter_context(tc.tile_pool(name="g", bufs=2))
        tpool = ctx.enter_context(tc.tile_pool(name="t", bufs=2))
        opool = ctx.enter_context(tc.tile_pool(name="o", bufs=2))

        out_v = out.ap().rearrange("p (q k) -> p q k", k=63)

        for b in [bb for _ in range(reps) for bb in range(nbatch)]:
            gs = []
            for j in range(3):
                g = gpool.tile([128, m, ES], f16, tag=f"g{j}")
                nc.gpsimd.dma_gather(
                    g[:], pts[j].ap()[:],
                    idx128[:, j * lw + b * (nb // 16): j * lw + (b + 1) * (nb // 16)],
                    nb, nb, ES)
                gs.append(g)
            ss = []
            for j in range(3):
                wb = (wt16[:, j * mc + b * m: j * mc + (b + 1) * m]
                      .unsqueeze(2).to_broadcast([128, m, 192]))
                mt = tpool.tile([128, m, 192], f16, tag="mt", bufs=3)
                nc.vector.tensor_tensor(mt[:], gs[j][:, :, 192:384], wb, A.mult)
                st = tpool.tile([128, m, 192], f16, tag=f"s{j}")
                nc.vector.tensor_tensor(st[:], mt[:], gs[j][:, :, 0:192], A.add)
                ss.append(st)
            p01 = tpool.tile([128, m, 192], f16, tag="p01")
            nc.vector.tensor_tensor(p01[:], ss[0][:], ss[1][:], A.mult)
            pr = tpool.tile([128, m, 192], f16, tag="pr")
            nc.vector.tensor_tensor(pr[:], p01[:], ss[2][:], A.mult)
            delta = tpool.tile([128, m, 3], f32, tag="delta")
            nc.vector.tensor_reduce(
                delta[:], pr[:].rearrange("p q (f c) -> p q f c", c=CHAN),
                mybir.AxisListType.X, A.add)
            pxyz = tpool.tile([128, m, 3], f32, tag="pxyz")
            nc.vector.tensor_tensor(
                pxyz[:], delta[:], xyz[:, b * m:(b + 1) * m, :], A.add)

            ot = opool.tile([128, m, 63], f32, tag="ot")
            nc.scalar.activation(ot[:, :, 0:3], pxyz[:], ACT.Copy)
            # base frequency: sin(p) and cos(p) = sin(pi/2 - p) on the ACT
            # engine (p is in (-0.2, 1.2), well inside the [-pi, pi] domain)
            nc.scalar.activation(ot[:, :, 3:6], pxyz[:], ACT.Sin)
            nc.scalar.activation(ot[:, :, 6:9], pxyz[:], ACT.Sin,
                                 bias=hpi[:], scale=-1.0)
            # frequency doubling: s' = 2sc, c' = 1 - 2s^2
            for k in range(1, FREQ):
                sm1 = ot[:, :, 6 * k - 3:6 * k]
                cm1 = ot[:, :, 6 * k:6 * k + 3]
                nc.vector.scalar_tensor_tensor(
                    ot[:, :, 6 * k + 3:6 * k + 6], sm1, 2.0, cm1,
                    A.mult, A.mult)
                sq = tpool.tile([128, m, 3], f32, tag="sq")
                nc.vector.tensor_tensor(sq[:], sm1, sm1, A.mult)
                nc.vector.tensor_scalar(
                    ot[:, :, 6 * k + 6:6 * k + 9], sq[:], -2.0, 1.0,
                    A.mult, A.add)

            nc.sync.dma_start(out_v[:, b * m:(b + 1) * m, :], ot[:])

    nc.compile()
    return nc


def pack_tables(feat0, feat1, feat2):
    """Build the 3 per-component gather tables (NROWS, 384) fp16 [A | D]."""
    pts = []
    for j in range(3):
        planes = np.stack([feat0[j], feat1[j], feat2[j]], axis=0)  # (3,64,100,256)
        a = np.ascontiguousarray(
            planes.transpose(2, 3, 0, 1).reshape(NROWS, 3 * CHAN)).astype(np.float32)
        d = np.zeros_like(a)
        d[:-1] = a[1:] - a[:-1]
        d[RESO - 1::RESO] = 0.0  # x=255 rows never used as base; avoid cross-frame
        pts.append(np.concatenate([a, d], axis=1).astype(np.float16))
    return pts


def pack_x(x_shard):
    """x_shard (PC,4) -> (xw (16,4*LW), xc (128,MC*4))."""
    pc = x_shard.shape[0]
    lw, mc = pc // 16, pc // 128
    xw = np.concatenate(
        [np.ascontiguousarray(x_shard[:, j].reshape(lw, 16).T) for j in range(4)],
        axis=1).astype(np.float32)
    xc = np.ascontiguousarray(
        x_shard.reshape(mc, 128, 4).transpose(1, 0, 2).reshape(128, mc * 4)
    ).astype(np.float32)
    return xw, xc


_NC_CACHE = {}


def kernel(x, feat0, feat1, feat2):
    x = np.asarray(x, dtype=np.float32)
    feat0 = np.asarray(feat0, dtype=np.float32)
    feat1 = np.asarray(feat1, dtype=np.float32)
    feat2 = np.asarray(feat2, dtype=np.float32)

    if "nc" not in _NC_CACHE:
        _NC_CACHE["nc"] = build_program()
    nc = _NC_CACHE["nc"]

    pts = pack_tables(feat0, feat1, feat2)

    in_maps = []
    for k in range(NCORES):
        xw, xc = pack_x(x[k * PC:(k + 1) * PC])
        in_maps.append({
            "xw": xw, "xc": xc,
            "pt0": pts[0], "pt1": pts[1], "pt2": pts[2],
        })

    res = run_bass_kernel_spmd(nc, in_maps, core_ids=list(range(NCORES)))
    outs = []
    for k in range(NCORES):
        o = res.results[k]["out"].reshape(128, PC // 128, 63)
        outs.append(o.transpose(1, 0, 2).reshape(PC, 63))
    return np.concatenate(outs, axis=0)


# revision 6
# speedup vs baseline: 1.1881x; 1.0429x over previous
"""DNeRF-TensoRF sampler kernel for Trainium2 (8 NeuronCores, data-parallel over points).

Strategy:
  - Host pre-packs the 9 feature planes (3 feats x 3 components) into 3 gather
    tables PT_j of shape (NUM_FRAMES*RESO, 384) fp16 where row r = t*RESO+x holds
    [A | D]: A[f*64+c] = feat_f[j,c,t,x], D = A(next x) - A  (difference table, so
    the device lerp is A + wx*D).  The y (time) coordinate is an exact integer
    frame index, so the bilinear reduces to a 1-D lerp along x at row t.
  - x is sharded over 8 cores along the point axis.  Each core processes
    PC = P/8 points: computes int16 row indices on 16 partitions (the layout
    dma_gather needs), gathers 768B fp16 rows per (point, component) with SWDGE
    dma_gather, lerps/multiplies in fp16, reduces over 64 channels in fp32, and
    evaluates the sin/cos positional encoding via one ACT-engine sin/cos pair
    at the base frequency plus a double-angle recurrence on the vector engine
    (the FREQ frequencies are exactly 2^k, so sin/cos at level k come from
    level k-1 with 3 small ops instead of a full range reduction).
  - Outputs are written in an interleaved device layout and un-permuted on host.
"""
import sys

sys.path.insert(0, "/opt/trn_rl_repo")

from contextlib import ExitStack

import numpy as np

import concourse.bacc as bacc
import concourse.bass as bass
import concourse.mybir as mybir
import concourse.tile as tile
from concourse.bass_utils import run_bass_kernel_spmd

NUM_FRAMES = 100
RESO = 256
CHAN = 64
FREQ = 10
P = 524288
NCORES = 8
PC = P // NCORES            # 65536 points per core
NROWS = NUM_FRAMES * RESO   # 25600 table rows
ES = 2 * 3 * CHAN           # 384 fp16 per gather row ([A|D], 3 feats x 64 ch)
NB = 1024                   # points per gather batch

HALF_PI = float(np.pi / 2)


def build_program(pc=PC, nb=NB, hw_round=True, reps=1):
    """hw_round: HW casts fp32->int with round-to-nearest; CoreSim truncates.
    floor(v) is computed as cast(v + CAST_OFF) with CAST_OFF=-0.5 on HW, 0 in
    sim.  reps: repeat the main loop (timing amplification)."""
    cast_off = -0.5 if hw_round else 0.0
    m = nb // 128           # free slots per batch
    lw = pc // 16           # idx-layout free size
    mc = pc // 128          # interleaved-layout free size
    nbatch = pc // nb
    f32 = mybir.dt.float32
    f16 = mybir.dt.float16
    i16 = mybir.dt.int16
    i32 = mybir.dt.int32
    A = mybir.AluOpType
    ACT = mybir.ActivationFunctionType

    nc = bacc.Bacc("TRN2", target_bir_lowering=False, debug=False)

    xw = nc.dram_tensor("xw", [16, 4 * lw], f32, kind="ExternalInput")
    xc = nc.dram_tensor("xc", [128, mc * 4], f32, kind="ExternalInput")
    pts = [
        nc.dram_tensor(f"pt{j}", [NROWS, ES], f16, kind="ExternalInput")
        for j in range(3)
    ]
    out = nc.dram_tensor("out", [128, mc * 63], f32, kind="ExternalOutput")

    with tile.TileContext(nc) as tc, ExitStack() as ctx:
        cpool = ctx.enter_context(tc.tile_pool(name="const", bufs=1))
        hpi = cpool.tile([128, 1], f32)
        nc.vector.memset(hpi[:], HALF_PI)
        pxall = cpool.tile([128, mc, 3], f32)   # staged p = xyz + delta
        xct = cpool.tile([128, mc * 4], f32)
        nc.sync.dma_start(xct[:], xc.ap()[:])
        wxt = cpool.tile([128, 3 * mc], f32)
        wt16 = cpool.tile([128, 3 * mc], f16)
        idx128 = cpool.tile([128, 3 * lw], i16)

        # ---- setup: per-point row indices on 16 partitions, weights on 128 ----
        with tc.tile_pool(name="setup", bufs=1) as spool:
            xwt = spool.tile([16, 4 * lw], f32)
            nc.sync.dma_start(xwt[:], xw.ap()[:])
            xw_t = xwt[:, 3 * lw:4 * lw]
            for j in range(3):
                xw_j = xwt[:, j * lw:(j + 1) * lw]
                ixw = spool.tile([16, lw], f32, tag="ixw")
                nc.vector.tensor_scalar(ixw[:], xw_j, 255.0, cast_off,
                                        A.mult, A.add)
                nc.vector.tensor_scalar(ixw[:], ixw[:], 0.0, None, A.max)
                ixi = spool.tile([16, lw], i32, tag="ixi")
                nc.vector.tensor_copy(ixi[:], ixw[:])   # == floor(255*x)
                x0w = spool.tile([16, lw], f32, tag="x0w")
                nc.vector.tensor_copy(x0w[:], ixi[:])
                rw = spool.tile([16, lw], f32, tag="rw")
                nc.vector.scalar_tensor_tensor(
                    rw[:], xw_t, 256.0, x0w[:], A.mult, A.add)
                idx16 = spool.tile([16, lw], i16, tag="idx16")
                nc.vector.tensor_copy(idx16[:], rw[:])
                for k in range(8):
                    nc.sync.dma_start(
                        idx128[16 * k:16 * (k + 1), j * lw:(j + 1) * lw], idx16[:])
                # interleaved-layout lerp weights: wx1 = ix - floor(ix)
                xj = xct[:].rearrange("p (q f) -> p q f", f=4)[:, :, j]
                ixc = spool.tile([128, mc], f32, tag="ixc")
                nc.vector.tensor_scalar(ixc[:], xj, 255.0, None, A.mult)
                icm = spool.tile([128, mc], f32, tag="icm")
                nc.vector.tensor_scalar(icm[:], xj, 255.0, cast_off,
                                        A.mult, A.add)
                nc.vector.tensor_scalar(icm[:], icm[:], 0.0, None, A.max)
                ici = spool.tile([128, mc], i32, tag="ici")
                nc.vector.tensor_copy(ici[:], icm[:])
                icf = spool.tile([128, mc], f32, tag="icf")
                nc.vector.tensor_copy(icf[:], ici[:])
                nc.vector.tensor_tensor(
                    wxt[:, j * mc:(j + 1) * mc], ixc[:], icf[:], A.subtract)
            nc.vector.tensor_copy(wt16[:], wxt[:])  # fp16 weights for the lerp

        xyz = xct[:].rearrange("p (q f) -> p q f", f=4)[:, :, 0:3]  # (128, mc, 3)

        gpool = ctx.enter_context(tc.tile_pool(name="g", bufs=2))
        tpool = ctx.enter_context(tc.tile_pool(name="t", bufs=2))
        opool = ctx.enter_context(tc.tile_pool(name="o", bufs=2))

        out_v = out.ap().rearrange("p (q k) -> p q k", k=63)

        def encode_chunk(c0, cw):
            """Positional encoding for staged points pxall[:, c0:c0+cw, :]."""
            px = pxall[:, c0:c0 + cw, :]
            ot = opool.tile([128, cw, 63], f32, tag="ot")
            nc.scalar.activation(ot[:, :, 0:3], px, ACT.Copy)
            # base frequency: sin(p) and cos(p) = sin(pi/2 - p) on the ACT
            # engine (p is in (-0.2, 1.2), well inside the [-pi, pi] domain)
            nc.scalar.activation(ot[:, :, 3:6], px, ACT.Sin)
            nc.scalar.activation(ot[:, :, 6:9], px, ACT.Sin,
                                 bias=hpi[:], scale=-1.0)
            # frequency doubling: s' = 2sc, c' = 1 - 2s^2
            for k in range(1, FREQ):
                sm1 = ot[:, :, 6 * k - 3:6 * k]
                cm1 = ot[:, :, 6 * k:6 * k + 3]
                nc.vector.scalar_tensor_tensor(
                    ot[:, :, 6 * k + 3:6 * k + 6], sm1, 2.0, cm1,
                    A.mult, A.mult)
                sq = tpool.tile([128, cw, 3], f32, tag="sq")
                nc.vector.tensor_tensor(sq[:], sm1, sm1, A.mult)
                nc.vector.tensor_scalar(
                    ot[:, :, 6 * k + 6:6 * k + 9], sq[:], -2.0, 1.0,
                    A.mult, A.add)
            nc.sync.dma_start(out_v[:, c0:c0 + cw, :], ot[:])

        EB = 8                       # batches per encoding chunk
        for b in [bb for _ in range(reps) for bb in range(nbatch)]:
            gs = []
            for j in range(3):
                g = gpool.tile([128, m, ES], f16, tag=f"g{j}")
                nc.gpsimd.dma_gather(
                    g[:], pts[j].ap()[:],
                    idx128[:, j * lw + b * (nb // 16): j * lw + (b + 1) * (nb // 16)],
                    nb, nb, ES)
                gs.append(g)
            ss = []
            for j in range(3):
                wb = (wt16[:, j * mc + b * m: j * mc + (b + 1) * m]
                      .unsqueeze(2).to_broadcast([128, m, 192]))
                mt = tpool.tile([128, m, 192], f16, tag="mt", bufs=3)
                nc.vector.tensor_tensor(mt[:], gs[j][:, :, 192:384], wb, A.mult)
                st = tpool.tile([128, m, 192], f16, tag=f"s{j}")
                nc.vector.tensor_tensor(st[:], mt[:], gs[j][:, :, 0:192], A.add)
                ss.append(st)
            p01 = tpool.tile([128, m, 192], f16, tag="p01")
            nc.vector.tensor_tensor(p01[:], ss[0][:], ss[1][:], A.mult)
            pr = tpool.tile([128, m, 192], f16, tag="pr")
            nc.vector.tensor_tensor(pr[:], p01[:], ss[2][:], A.mult)
            delta = tpool.tile([128, m, 3], f32, tag="delta")
            nc.vector.tensor_reduce(
                delta[:], pr[:].rearrange("p q (f c) -> p q f c", c=CHAN),
                mybir.AxisListType.X, A.add)
            nc.vector.tensor_tensor(
                pxall[:, b * m:(b + 1) * m, :], delta[:],
                xyz[:, b * m:(b + 1) * m, :], A.add)

            if reps == 1 and b % EB == EB - 1:
                encode_chunk((b - EB + 1) * m, EB * m)

    nc.compile()
    return nc


def pack_tables(feat0, feat1, feat2):
    """Build the 3 per-component gather tables (NROWS, 384) fp16 [A | D]."""
    pts = []
    for j in range(3):
        planes = np.stack([feat0[j], feat1[j], feat2[j]], axis=0)  # (3,64,100,256)
        a = np.ascontiguousarray(
            planes.transpose(2, 3, 0, 1).reshape(NROWS, 3 * CHAN)).astype(np.float32)
        d = np.zeros_like(a)
        d[:-1] = a[1:] - a[:-1]
        d[RESO - 1::RESO] = 0.0  # x=255 rows never used as base; avoid cross-frame
        pts.append(np.concatenate([a, d], axis=1).astype(np.float16))
    return pts


def pack_x(x_shard):
    """x_shard (PC,4) -> (xw (16,4*LW), xc (128,MC*4))."""
    pc = x_shard.shape[0]
    lw, mc = pc // 16, pc // 128
    xw = np.concatenate(
        [np.ascontiguousarray(x_shard[:, j].reshape(lw, 16).T) for j in range(4)],
        axis=1).astype(np.float32)
    xc = np.ascontiguousarray(
        x_shard.reshape(mc, 128, 4).transpose(1, 0, 2).reshape(128, mc * 4)
    ).astype(np.float32)
    return xw, xc


_NC_CACHE = {}


def kernel(x, feat0, feat1, feat2):
    x = np.asarray(x, dtype=np.float32)
    feat0 = np.asarray(feat0, dtype=np.float32)
    feat1 = np.asarray(feat1, dtype=np.float32)
    feat2 = np.asarray(feat2, dtype=np.float32)

    if "nc" not in _NC_CACHE:
        _NC_CACHE["nc"] = build_program()
    nc = _NC_CACHE["nc"]

    pts = pack_tables(feat0, feat1, feat2)

    in_maps = []
    for k in range(NCORES):
        xw, xc = pack_x(x[k * PC:(k + 1) * PC])
        in_maps.append({
            "xw": xw, "xc": xc,
            "pt0": pts[0], "pt1": pts[1], "pt2": pts[2],
        })

    res = run_bass_kernel_spmd(nc, in_maps, core_ids=list(range(NCORES)))
    outs = []
    for k in range(NCORES):
        o = res.results[k]["out"].reshape(128, PC // 128, 63)
        outs.append(o.transpose(1, 0, 2).reshape(PC, 63))
    return np.concatenate(outs, axis=0)


# revision 20
# speedup vs baseline: 2.0634x; 1.7367x over previous
"""DNeRF-TensoRF sampler kernel for Trainium2 (8 NeuronCores, data-parallel over points).

Strategy:
  - Host pre-packs the 9 feature planes (3 feats x 3 components) into 3 gather
    tables PT_j of shape (NUM_FRAMES*RESO, 384) fp16 where row r = t*RESO+x holds
    [A | D]: A[f*64+c] = feat_f[j,c,t,x], D = A(next x) - A  (difference table, so
    the device lerp is A + wx*D).  The y (time) coordinate is an exact integer
    frame index, so the bilinear reduces to a 1-D lerp along x at row t.
  - x is sharded over 8 cores along the point axis.  Each core processes
    PC = P/8 points: computes int16 row indices on 16 partitions (the layout
    dma_gather needs), gathers 768B fp16 rows per (point, component) with SWDGE
    dma_gather, lerps/multiplies in fp16, reduces over 64 channels in fp32, and
    evaluates the sin/cos positional encoding via one ACT-engine sin/cos pair
    at the base frequency plus a double-angle recurrence on the vector engine
    (the FREQ frequencies are exactly 2^k, so sin/cos at level k come from
    level k-1 with 3 small ops instead of a full range reduction).
  - Outputs are written in an interleaved device layout and un-permuted on host.
"""
import sys

sys.path.insert(0, "/opt/trn_rl_repo")

from contextlib import ExitStack

import numpy as np

import concourse.bacc as bacc
import concourse.bass as bass
import concourse.mybir as mybir
import concourse.tile as tile
from concourse.bass_utils import run_bass_kernel_spmd

NUM_FRAMES = 100
RESO = 256
CHAN = 64
FREQ = 10
P = 524288
NCORES = 8
PC = P // NCORES            # 65536 points per core
NROWS = NUM_FRAMES * RESO   # 25600 table rows
ES = 2 * 3 * CHAN           # 384 fp16 per gather row ([A|D], 3 feats x 64 ch)
NB = 1024                   # points per gather batch

HALF_PI = float(np.pi / 2)


def build_program(pc=PC, nb=NB, hw_round=True, reps=1):
    """hw_round: HW casts fp32->int with round-to-nearest; CoreSim truncates.
    floor(v) is computed as cast(v + CAST_OFF) with CAST_OFF=-0.5 on HW, 0 in
    sim.  reps: repeat the main loop (timing amplification)."""
    cast_off = -0.5 if hw_round else 0.0
    m = nb // 128           # free slots per batch
    lw = pc // 16           # idx-layout free size
    mc = pc // 128          # interleaved-layout free size
    nbatch = pc // nb
    f32 = mybir.dt.float32
    f16 = mybir.dt.float16
    i16 = mybir.dt.int16
    i32 = mybir.dt.int32
    A = mybir.AluOpType
    ACT = mybir.ActivationFunctionType

    nc = bacc.Bacc("TRN2", target_bir_lowering=False, debug=False,
                   num_swdge_queues=4)

    xw = nc.dram_tensor("xw", [16, 4 * lw], f32, kind="ExternalInput")
    xc = nc.dram_tensor("xc", [128, mc * 4], f32, kind="ExternalInput")
    pts = [
        nc.dram_tensor(f"pt{j}", [NROWS, ES], f16, kind="ExternalInput")
        for j in range(3)
    ]
    out = nc.dram_tensor("out", [128, mc * 63], f32, kind="ExternalOutput")

    with tile.TileContext(nc) as tc, ExitStack() as ctx:
        cpool = ctx.enter_context(tc.tile_pool(name="const", bufs=1))
        hpi = cpool.tile([128, 1], f32)
        nc.vector.memset(hpi[:], HALF_PI)
        pxall = cpool.tile([128, mc, 3], f32)   # staged p = xyz + delta
        xct = cpool.tile([128, mc * 4], f32)
        nc.sync.dma_start(xct[:], xc.ap()[:])
        wxt = cpool.tile([128, 3 * mc], f32)
        wt16 = cpool.tile([128, 3 * mc], f16)
        idx128 = cpool.tile([128, 3 * lw], i16)

        # ---- setup: per-point row indices on 16 partitions, weights on 128 ----
        with tc.tile_pool(name="setup", bufs=1) as spool:
            xwt = spool.tile([16, 4 * lw], f32)
            nc.sync.dma_start(xwt[:], xw.ap()[:])
            xw_t = xwt[:, 3 * lw:4 * lw]
            for j in range(3):
                xw_j = xwt[:, j * lw:(j + 1) * lw]
                ixw = spool.tile([16, lw], f32, tag="ixw")
                nc.vector.tensor_scalar(ixw[:], xw_j, 255.0, cast_off,
                                        A.mult, A.add)
                nc.vector.tensor_scalar(ixw[:], ixw[:], 0.0, None, A.max)
                ixi = spool.tile([16, lw], i32, tag="ixi")
                nc.vector.tensor_copy(ixi[:], ixw[:])   # == floor(255*x)
                x0w = spool.tile([16, lw], f32, tag="x0w")
                nc.vector.tensor_copy(x0w[:], ixi[:])
                rw = spool.tile([16, lw], f32, tag="rw")
                nc.vector.scalar_tensor_tensor(
                    rw[:], xw_t, 256.0, x0w[:], A.mult, A.add)
                idx16 = spool.tile([16, lw], i16, tag="idx16")
                nc.vector.tensor_copy(idx16[:], rw[:])
                for k in range(8):
                    nc.sync.dma_start(
                        idx128[16 * k:16 * (k + 1), j * lw:(j + 1) * lw], idx16[:])
                # interleaved-layout lerp weights: wx1 = ix - floor(ix)
                xj = xct[:].rearrange("p (q f) -> p q f", f=4)[:, :, j]
                ixc = spool.tile([128, mc], f32, tag="ixc")
                nc.vector.tensor_scalar(ixc[:], xj, 255.0, None, A.mult)
                icm = spool.tile([128, mc], f32, tag="icm")
                nc.vector.tensor_scalar(icm[:], xj, 255.0, cast_off,
                                        A.mult, A.add)
                nc.vector.tensor_scalar(icm[:], icm[:], 0.0, None, A.max)
                ici = spool.tile([128, mc], i32, tag="ici")
                nc.vector.tensor_copy(ici[:], icm[:])
                icf = spool.tile([128, mc], f32, tag="icf")
                nc.vector.tensor_copy(icf[:], ici[:])
                nc.vector.tensor_tensor(
                    wxt[:, j * mc:(j + 1) * mc], ixc[:], icf[:], A.subtract)
            nc.vector.tensor_copy(wt16[:], wxt[:])  # fp16 weights for the lerp

        xyz = xct[:].rearrange("p (q f) -> p q f", f=4)[:, :, 0:3]  # (128, mc, 3)

        gpool = ctx.enter_context(tc.tile_pool(name="g", bufs=2))
        tpool = ctx.enter_context(tc.tile_pool(name="t", bufs=2))
        opool = ctx.enter_context(tc.tile_pool(name="o", bufs=2))

        out_v = out.ap().rearrange("p (q k) -> p q k", k=63)

        def encode_chunk(c0, cw):
            """Positional encoding for staged points pxall[:, c0:c0+cw, :]."""
            px = pxall[:, c0:c0 + cw, :]
            ot = opool.tile([128, cw, 63], f32, tag="ot")
            nc.scalar.activation(ot[:, :, 0:3], px, ACT.Copy)
            # base frequency: sin(p) and cos(p) = sin(pi/2 - p) on the ACT
            # engine (p is in (-0.2, 1.2), well inside the [-pi, pi] domain)
            nc.scalar.activation(ot[:, :, 3:6], px, ACT.Sin)
            nc.scalar.activation(ot[:, :, 6:9], px, ACT.Sin,
                                 bias=hpi[:], scale=-1.0)
            # frequency doubling: s' = 2sc, c' = 1 - 2s^2
            for k in range(1, FREQ):
                sm1 = ot[:, :, 6 * k - 3:6 * k]
                cm1 = ot[:, :, 6 * k:6 * k + 3]
                nc.vector.scalar_tensor_tensor(
                    ot[:, :, 6 * k + 3:6 * k + 6], sm1, 2.0, cm1,
                    A.mult, A.mult)
                sq = tpool.tile([128, cw, 3], f32, tag="sq")
                nc.vector.tensor_tensor(sq[:], sm1, sm1, A.mult)
                nc.vector.tensor_scalar(
                    ot[:, :, 6 * k + 6:6 * k + 9], sq[:], -2.0, 1.0,
                    A.mult, A.add)
            nc.sync.dma_start(out_v[:, c0:c0 + cw, :], ot[:])

        EB = 8                       # batches per encoding chunk
        for b in [bb for _ in range(reps) for bb in range(nbatch)]:
            gs = []
            for j in range(3):
                g = gpool.tile([128, m, ES], f16, tag=f"g{j}")
                nc.gpsimd.dma_gather(
                    g[:], pts[j].ap()[:],
                    idx128[:, j * lw + b * (nb // 16): j * lw + (b + 1) * (nb // 16)],
                    nb, nb, ES, queue_num=(b * 3 + j) % 4)
                gs.append(g)
            ss = []
            for j in range(3):
                wb = (wt16[:, j * mc + b * m: j * mc + (b + 1) * m]
                      .unsqueeze(2).to_broadcast([128, m, 192]))
                mt = tpool.tile([128, m, 192], f16, tag="mt", bufs=3)
                nc.vector.tensor_tensor(mt[:], gs[j][:, :, 192:384], wb, A.mult)
                st = tpool.tile([128, m, 192], f16, tag=f"s{j}")
                nc.vector.tensor_tensor(st[:], mt[:], gs[j][:, :, 0:192], A.add)
                ss.append(st)
            p01 = tpool.tile([128, m, 192], f16, tag="p01")
            nc.vector.tensor_tensor(p01[:], ss[0][:], ss[1][:], A.mult)
            pr = tpool.tile([128, m, 192], f16, tag="pr")
            nc.vector.tensor_tensor(pr[:], p01[:], ss[2][:], A.mult)
            delta = tpool.tile([128, m, 3], f32, tag="delta")
            nc.vector.tensor_reduce(
                delta[:], pr[:].rearrange("p q (f c) -> p q f c", c=CHAN),
                mybir.AxisListType.X, A.add)
            nc.vector.tensor_tensor(
                pxall[:, b * m:(b + 1) * m, :], delta[:],
                xyz[:, b * m:(b + 1) * m, :], A.add)

            if b % EB == EB - 1:
                encode_chunk((b - EB + 1) * m, EB * m)

    nc.compile()
    return nc


def pack_tables(feat0, feat1, feat2):
    """Build the 3 per-component gather tables (NROWS, 384) fp16 [A | D]."""
    pts = []
    for j in range(3):
        planes = np.stack([feat0[j], feat1[j], feat2[j]], axis=0)  # (3,64,100,256)
        a = np.ascontiguousarray(
            planes.transpose(2, 3, 0, 1).reshape(NROWS, 3 * CHAN)).astype(np.float32)
        d = np.zeros_like(a)
        d[:-1] = a[1:] - a[:-1]
        d[RESO - 1::RESO] = 0.0  # x=255 rows never used as base; avoid cross-frame
        pts.append(np.concatenate([a, d], axis=1).astype(np.float16))
    return pts


def pack_x(x_shard):
    """x_shard (PC,4) -> (xw (16,4*LW), xc (128,MC*4))."""
    pc = x_shard.shape[0]
    lw, mc = pc // 16, pc // 128
    xw = np.concatenate(
        [np.ascontiguousarray(x_shard[:, j].reshape(lw, 16).T) for j in range(4)],
        axis=1).astype(np.float32)
    xc = np.ascontiguousarray(
        x_shard.reshape(mc, 128, 4).transpose(1, 0, 2).reshape(128, mc * 4)
    ).astype(np.float32)
    return xw, xc


_NC_CACHE = {}


def kernel(x, feat0, feat1, feat2):
    x = np.asarray(x, dtype=np.float32)
    feat0 = np.asarray(feat0, dtype=np.float32)
    feat1 = np.asarray(feat1, dtype=np.float32)
    feat2 = np.asarray(feat2, dtype=np.float32)

    if "nc" not in _NC_CACHE:
        _NC_CACHE["nc"] = build_program()
    nc = _NC_CACHE["nc"]

    pts = pack_tables(feat0, feat1, feat2)

    in_maps = []
    for k in range(NCORES):
        xw, xc = pack_x(x[k * PC:(k + 1) * PC])
        in_maps.append({
            "xw": xw, "xc": xc,
            "pt0": pts[0], "pt1": pts[1], "pt2": pts[2],
        })

    res = run_bass_kernel_spmd(nc, in_maps, core_ids=list(range(NCORES)))
    outs = []
    for k in range(NCORES):
        o = res.results[k]["out"].reshape(128, PC // 128, 63)
        outs.append(o.transpose(1, 0, 2).reshape(PC, 63))
    return np.concatenate(outs, axis=0)
